# revision 2
# baseline (speedup 1.0000x reference)
"""Trainium2 Bass kernel for nn_DiscriminationLoss (segment_reduce).

Strategy (8 NeuronCores, pixel-sharded):
  - Each core gets 1/8 of the 4M pixels: pred slice [8, 524288] f32 and
    labels slice [524288] i32.
  - On-chip, pixels are tiled [128 partitions x F free]. For each free
    column t (a "block" of 128 pixels), the DVE builds a one-hot matrix
    oh[p, j] = (labels[p, t] == j+1) for j in 0..31 (label 0 = background
    is dropped, matching the reference which discards segment 0).
    One-hot generation is batched over FC blocks in a single
    tensor_tensor(is_equal) with broadcast access patterns.
  - The PE contracts each block: psum[9, 32] += pred9[128, 9]^T @ oh[128, 32]
    where pred9 = 8 channels + a ones column (the ones column yields the
    per-label pixel counts N_k). PSUM accumulates across all 4096 blocks.
  - Each core emits S_partial [9, 32]. The host sums partials over cores
    (the "psum" step) and evaluates the tiny O(K^2) pairwise tail in f64.
"""

import sys
import functools

sys.path.insert(0, "/opt/trn_rl_repo")

import numpy as np

C = 8
K = 32
NCORES = 8
H = W = 2048
PTOT = H * W
PCORE = PTOT // NCORES  # 524288
NCH = C + 1  # 8 channels + ones column (counts)
SIGMA_DIS = 3.0

FG = 1024  # free-dim length per DMA group (128*FG pixels per group)
FC = 256   # free-dim length per one-hot chunk (blocks per tensor_tensor)


def build_nc(pcore=PCORE, fg=FG, fc=FC, gps_mod=0):
    """Build the per-core Bass program (SPMD across 8 cores).

    gps_mod: if >0, every gps_mod-th one-hot chunk runs on GpSimd
    instead of VectorE (engine load balancing).
    """
    import concourse.bacc as bacc
    import concourse.tile as tile
    import concourse.mybir as mybir
    from contextlib import ExitStack

    ftot = pcore // 128
    assert pcore % 128 == 0
    fg = min(fg, ftot)
    assert ftot % fg == 0 and fg % fc == 0
    ngroups = ftot // fg
    f32 = mybir.dt.float32
    i32 = mybir.dt.int32

    nc = bacc.Bacc(
        "TRN2", target_bir_lowering=False, debug=False, num_devices=NCORES
    )
    pred_ext = nc.dram_tensor("pred", [C, pcore], f32, kind="ExternalInput")
    lab_ext = nc.dram_tensor("labels", [pcore], i32, kind="ExternalInput")
    iota_ext = nc.dram_tensor("iota", [128, K], i32, kind="ExternalInput")
    out_ext = nc.dram_tensor("out_s", [NCH, K], f32, kind="ExternalOutput")

    with tile.TileContext(nc) as tc, ExitStack() as ctx:
        const_pool = ctx.enter_context(tc.tile_pool(name="const", bufs=1))
        slab_pool = ctx.enter_context(tc.tile_pool(name="slab", bufs=2))
        lab_pool = ctx.enter_context(tc.tile_pool(name="lab", bufs=2))
        oh_pool = ctx.enter_context(tc.tile_pool(name="oh", bufs=2))
        psum_pool = ctx.enter_context(tc.tile_pool(name="psum", bufs=1, space="PSUM"))
        out_pool = ctx.enter_context(tc.tile_pool(name="outp", bufs=1))

        iota_t = const_pool.tile([128, K], i32)
        nc.sync.dma_start(iota_t[:], iota_ext[:])

        psum_t = psum_pool.tile([NCH, K], f32)

        nblocks = ftot
        blk = 0
        chunk_idx = 0
        for g in range(ngroups):
            gpx = 128 * fg  # pixels per group
            slab = slab_pool.tile([128, NCH * fg], f32)
            nc.sync.dma_start(
                slab[:, : C * fg].rearrange("p (c f) -> p c f", c=C),
                pred_ext[:, g * gpx : (g + 1) * gpx].rearrange(
                    "c (p f) -> p c f", p=128
                ),
            )
            nc.gpsimd.memset(slab[:, C * fg :], 1.0)
            ltile = lab_pool.tile([128, fg], i32)
            nc.sync.dma_start(
                ltile[:],
                lab_ext[g * gpx : (g + 1) * gpx].rearrange("(p f) -> p f", p=128),
            )
            slab_r = slab[:].rearrange("p (c f) -> p c f", c=NCH)
            for ci in range(fg // fc):
                oh = oh_pool.tile([128, fc * K], f32)
                oh_r = oh[:].rearrange("p (t j) -> p t j", j=K)
                in0 = (
                    ltile[:, ci * fc : (ci + 1) * fc]
                    .unsqueeze(2)
                    .broadcast_to([128, fc, K])
                )
                in1 = iota_t[:].unsqueeze(1).broadcast_to([128, fc, K])
                if gps_mod and (chunk_idx % gps_mod == gps_mod - 1):
                    eng = nc.gpsimd
                else:
                    eng = nc.vector
                eng.tensor_tensor(oh_r, in0, in1, mybir.AluOpType.is_equal)
                chunk_idx += 1
                for t in range(fc):
                    tcol = ci * fc + t
                    nc.tensor.matmul(
                        psum_t[:],
                        slab_r[:, :, tcol],
                        oh_r[:, t, :],
                        start=(blk == 0),
                        stop=(blk == nblocks - 1),
                    )
                    blk += 1

        outt = out_pool.tile([NCH, K], f32)
        nc.vector.tensor_copy(outt[:], psum_t[:])
        nc.sync.dma_start(out_ext[:], outt[:])
    nc.compile()
    return nc


def make_iota_np():
    return np.broadcast_to(
        np.arange(1, K + 1, dtype=np.int32), (128, K)
    ).copy()


@functools.lru_cache(maxsize=1)
def _get_program():
    return build_nc()


def make_in_maps(pred_flat, labels_flat):
    iota_np = make_iota_np()
    in_maps = []
    for i in range(NCORES):
        sl = slice(i * PCORE, (i + 1) * PCORE)
        in_maps.append(
            {
                "pred": np.ascontiguousarray(pred_flat[:, sl]),
                "labels": np.ascontiguousarray(labels_flat[sl]),
                "iota": iota_np,
            }
        )
    return in_maps


def finish_host(parts, num_kernel):
    """parts: list of [NCH, K] per-core partials. Tiny O(K^2) tail in f64."""
    total = np.sum([p.astype(np.float64) for p in parts], axis=0)  # [9, 32]
    S = total[:C, :]  # [8, 32]
    N = total[C, :]  # [32]
    A = N * np.sum(S * S, axis=0)  # [32]
    kk = int(num_kernel)
    A = A[:kk]
    pair = A[:, None] + A[None, :]
    Dm = np.maximum(SIGMA_DIS - np.sqrt(pair), 0.0)
    term = np.log(Dm * Dm + 1.0)
    L = float(np.sum(np.triu(term, k=1)))
    L *= (kk - 1) / kk
    return np.float32(L)


_last_results = None


def kernel(pred_similarities, regions_mask, kernel_labels, num_kernel, **kw):
    global _last_results
    from concourse.bass_utils import run_bass_kernel_spmd

    pred_flat = np.asarray(pred_similarities, dtype=np.float32).reshape(C, PTOT)
    labels_flat = np.asarray(kernel_labels, dtype=np.int32).reshape(PTOT)

    nc = _get_program()
    in_maps = make_in_maps(pred_flat, labels_flat)
    res = run_bass_kernel_spmd(nc, in_maps, list(range(NCORES)))
    _last_results = res
    parts = [res.results[i]["out_s"] for i in range(NCORES)]
    return finish_host(parts, num_kernel)


# revision 7
# speedup vs baseline: 1.0158x; 1.0158x over previous
"""Trainium2 Bass kernel for nn_DiscriminationLoss (segment_reduce).

Strategy (8 NeuronCores, pixel-sharded):
  - Each core gets 1/8 of the 4M pixels: pred slice [8, 524288] f32 and
    labels slice [524288] i32.
  - Pixels are tiled [128 partitions x F free]. For each free column t
    (a "block" of 128 pixels), a one-hot matrix oh[p, j] = (labels[p,t]
    == j+1), j in 0..31 is built on DVE/GpSimd in bf16 (exact 0/1;
    label 0 = background is dropped, matching the reference). One-hot
    generation is batched over FC blocks per tensor_tensor(is_equal),
    laid out j-major so all access patterns are dense step-1 bf16
    (enables the DVE 2x perf mode).
  - pred is split exactly into two bf16 terms (hi = bf16(x),
    lo = bf16(x - hi)) so the PE can run bf16 matmuls (fp32 matmuls
    cost two PE passes) while keeping ~2^-17 relative precision.
  - Per block the PE contracts: psum[17, 32] += st[128, 17]^T @ oh[128, 32]
    where st = 8 hi channels | 8 lo channels | ones (ones -> counts N_k).
    PSUM accumulates over all blocks; optional 4-way column packing
    (tile_position) runs 4 blocks concurrently in the 128x128 array.
  - Each core emits [17, 32]. Host sums partials over cores (hi+lo and
    the psum step) and evaluates the tiny O(K^2) tail in f64.
"""

import sys
import functools

sys.path.insert(0, "/opt/trn_rl_repo")

import numpy as np

C = 8
K = 32
NCORES = 8
H = W = 2048
PTOT = H * W
PCORE = PTOT // NCORES  # 524288
SIGMA_DIS = 3.0

FG = 512   # free-dim length per DMA group (128*FG pixels per group)
FC = 256   # free-dim length per one-hot chunk (blocks per tensor_tensor)
USE_LO = True   # exact two-term bf16 split of pred
COL_PACK = 4    # concurrent col-groups in the PE array (1 or 4)
GPS_MOD = 3     # every GPS_MOD-th one-hot chunk runs on GpSimd


def build_nc(pcore=PCORE, fg=FG, fc=FC, use_lo=USE_LO, col_pack=COL_PACK,
             gps_mod=GPS_MOD):
    import concourse.bacc as bacc
    import concourse.tile as tile
    import concourse.mybir as mybir
    from contextlib import ExitStack

    ftot = pcore // 128
    assert pcore % 128 == 0
    fg = min(fg, ftot)
    assert ftot % fg == 0 and fg % fc == 0
    ngroups = ftot // fg
    f32 = mybir.dt.float32
    bf16 = mybir.dt.bfloat16
    i32 = mybir.dt.int32

    nch = 2 * C + 1 if use_lo else C + 1  # stationary width
    ones_col = nch - 1

    nc = bacc.Bacc(
        "TRN2", target_bir_lowering=False, debug=False, num_devices=NCORES
    )
    pred_ext = nc.dram_tensor("pred", [C, pcore], f32, kind="ExternalInput")
    lab_ext = nc.dram_tensor("labels", [pcore], i32, kind="ExternalInput")
    iota_ext = nc.dram_tensor("iotarep", [128, K * fc], bf16, kind="ExternalInput")
    out_ext = nc.dram_tensor("out_s", [col_pack * 32, K], f32, kind="ExternalOutput")

    with tile.TileContext(nc) as tc, ExitStack() as ctx:
        const_pool = ctx.enter_context(tc.tile_pool(name="const", bufs=1))
        slab32_pool = ctx.enter_context(tc.tile_pool(name="slab32", bufs=2))
        slabb_pool = ctx.enter_context(tc.tile_pool(name="slabb", bufs=2))
        lab_pool = ctx.enter_context(tc.tile_pool(name="lab", bufs=2))
        labb_pool = ctx.enter_context(tc.tile_pool(name="labb", bufs=2))
        oh_pool = ctx.enter_context(tc.tile_pool(name="oh", bufs=3))
        psum_pool = ctx.enter_context(tc.tile_pool(name="psum", bufs=1, space="PSUM"))
        out_pool = ctx.enter_context(tc.tile_pool(name="outp", bufs=1))

        iota_t = const_pool.tile([128, K * fc], bf16)
        nc.sync.dma_start(iota_t[:], iota_ext[:])

        psum_t = psum_pool.tile([col_pack * 32, K], f32)

        nblocks = ftot
        blk = 0
        chunk_idx = 0
        for g in range(ngroups):
            gpx = 128 * fg
            slab32 = slab32_pool.tile([128, C * fg], f32)
            nc.sync.dma_start(
                slab32[:].rearrange("p (c f) -> p c f", c=C),
                pred_ext[:, g * gpx : (g + 1) * gpx].rearrange(
                    "c (p f) -> p c f", p=128
                ),
            )
            slabb = slabb_pool.tile([128, nch * fg], bf16)
            # hi = bf16(pred) on ScalarE (cast copy)
            nc.scalar.copy(slabb[:, : C * fg], slab32[:])
            if use_lo:
                # lo = bf16(pred - f32(hi)) on GpSimd (keeps DVE free for one-hots)
                nc.gpsimd.tensor_tensor(
                    slabb[:, C * fg : 2 * C * fg],
                    slab32[:],
                    slabb[:, : C * fg],
                    mybir.AluOpType.subtract,
                )
            nc.gpsimd.memset(slabb[:, ones_col * fg :], 1.0)

            ltile = lab_pool.tile([128, fg], i32)
            nc.sync.dma_start(
                ltile[:],
                lab_ext[g * gpx : (g + 1) * gpx].rearrange("(p f) -> p f", p=128),
            )
            lbt = labb_pool.tile([128, fg], bf16)
            nc.gpsimd.tensor_copy(lbt[:], ltile[:])

            slab_r = slabb[:].rearrange("p (c f) -> p c f", c=nch)
            for ci in range(fg // fc):
                oh = oh_pool.tile([128, K * fc], bf16)
                # j-major: oh[p, j*fc + t] = (labels[p, t] == j+1)
                oh_r = oh[:].rearrange("p (j t) -> p j t", j=K)
                in0 = (
                    lbt[:, ci * fc : (ci + 1) * fc]
                    .unsqueeze(1)
                    .broadcast_to([128, K, fc])
                )
                in1 = iota_t[:].rearrange("p (j t) -> p j t", j=K)
                # GpSimd rejects TT+is_equal, so all one-hots run on DVE
                nc.vector.tensor_tensor(oh_r, in0, in1, mybir.AluOpType.is_equal)
                chunk_idx += 1
                for t in range(fc):
                    tcol = ci * fc + t
                    grp = blk % col_pack
                    if col_pack > 1:
                        nc.tensor.matmul(
                            psum_t[32 * grp : 32 * grp + nch, :],
                            slab_r[:, :, tcol],
                            oh[:, t :: fc],
                            start=(blk < col_pack),
                            stop=(blk >= nblocks - col_pack),
                            tile_position=(0, 32 * grp),
                        )
                    else:
                        nc.tensor.matmul(
                            psum_t[:nch, :],
                            slab_r[:, :, tcol],
                            oh[:, t :: fc],
                            start=(blk == 0),
                            stop=(blk == nblocks - 1),
                        )
                    blk += 1

        outt = out_pool.tile([col_pack * 32, K], f32)
        nc.gpsimd.memset(outt[:], 0.0)
        for b in range(col_pack):
            nc.vector.tensor_copy(
                outt[32 * b : 32 * b + nch, :], psum_t[32 * b : 32 * b + nch, :]
            )
        nc.sync.dma_start(out_ext[:], outt[:])
    nc.compile()
    return nc


def make_iota_np(fc=FC):
    import ml_dtypes

    # j-major repeated iota: value j+1 at [p, j*fc + t]
    v = np.repeat(np.arange(1, K + 1, dtype=np.float32), fc)
    return np.broadcast_to(v, (128, K * fc)).astype(ml_dtypes.bfloat16)


@functools.lru_cache(maxsize=1)
def _get_program():
    return build_nc()


def make_in_maps(pred_flat, labels_flat):
    iota_np = make_iota_np()
    in_maps = []
    for i in range(NCORES):
        sl = slice(i * PCORE, (i + 1) * PCORE)
        in_maps.append(
            {
                "pred": np.ascontiguousarray(pred_flat[:, sl]),
                "labels": np.ascontiguousarray(labels_flat[sl]),
                "iotarep": iota_np,
            }
        )
    return in_maps


def finish_host(parts, num_kernel, use_lo=USE_LO, col_pack=COL_PACK):
    """parts: list of per-core [col_pack*32, K] partials. f64 tail."""
    total = np.sum([p.astype(np.float64) for p in parts], axis=0)
    nch = 2 * C + 1 if use_lo else C + 1
    # fold the col_pack strips
    total = sum(total[32 * b : 32 * b + nch, :] for b in range(col_pack))
    if use_lo:
        S = total[:C, :] + total[C : 2 * C, :]
        N = total[2 * C, :]
    else:
        S = total[:C, :]
        N = total[C, :]
    A = N * np.sum(S * S, axis=0)  # [32]
    kk = int(num_kernel)
    A = A[:kk]
    pair = A[:, None] + A[None, :]
    Dm = np.maximum(SIGMA_DIS - np.sqrt(pair), 0.0)
    term = np.log(Dm * Dm + 1.0)
    L = float(np.sum(np.triu(term, k=1)))
    L *= (kk - 1) / kk
    return np.float32(L)


_last_results = None


def kernel(pred_similarities, regions_mask, kernel_labels, num_kernel, **kw):
    global _last_results
    from concourse.bass_utils import run_bass_kernel_spmd

    pred_flat = np.asarray(pred_similarities, dtype=np.float32).reshape(C, PTOT)
    labels_flat = np.asarray(kernel_labels, dtype=np.int32).reshape(PTOT)

    nc = _get_program()
    in_maps = make_in_maps(pred_flat, labels_flat)
    res = run_bass_kernel_spmd(nc, in_maps, list(range(NCORES)))
    _last_results = res
    parts = [res.results[i]["out_s"] for i in range(NCORES)]
    return finish_host(parts, num_kernel)


# revision 8
# speedup vs baseline: 1.0682x; 1.0516x over previous
"""Trainium2 Bass kernel for nn_DiscriminationLoss (segment_reduce).

Strategy (8 NeuronCores, pixel-sharded):
  - Each core gets 1/8 of the 4M pixels: pred slice [8, 524288] f32 and
    labels slice [524288] i32.
  - Pixels are tiled [128 partitions x F free]. For each free column t
    (a "block" of 128 pixels), a one-hot matrix oh[p, j] = (labels[p,t]
    == j+1), j in 0..31 is built on DVE (label 0 = background dropped,
    as in the reference). One-hot generation is batched over FC blocks
    per tensor_tensor(is_equal), j-major so all access patterns are
    dense step-1 16-bit (DVE 2x perf mode, ~4.4us per 256-block chunk).
  - pred is scaled by 2^14 and cast to fp16 on ScalarE (the scale rides
    the activation's free affine). fp16 keeps ~2^-11 per-element error;
    the final loss error lands ~1e-5. The host unscales.
  - Per block the PE contracts: psum[9, 32] += st[128, 9]^T @ oh[128, 32]
    (st = 8 scaled-fp16 channels | ones; the ones column yields counts).
    PSUM accumulates over all 4096 blocks per core.
  - GpSimd does nothing: its SBUF port is shared with the DVE and any
    long GpSimd op blocks the DVE one-hot stream (measured 3-4x stalls).
  - Each core emits [9, 32]. Host sums partials over cores (the psum
    step) and evaluates the tiny O(K^2) pairwise tail in f64.
"""

import sys
import functools

sys.path.insert(0, "/opt/trn_rl_repo")

import numpy as np

C = 8
K = 32
NCORES = 8
H = W = 2048
PTOT = H * W
PCORE = PTOT // NCORES  # 524288
SIGMA_DIS = 3.0
PRED_SCALE = float(2.0**14)

FG = 512   # free-dim length per DMA group (128*FG pixels per group)
FC = 256   # free-dim length per one-hot chunk (blocks per tensor_tensor)


def build_nc(pcore=PCORE, fg=FG, fc=FC):
    import concourse.bacc as bacc
    import concourse.tile as tile
    import concourse.mybir as mybir
    from contextlib import ExitStack

    ftot = pcore // 128
    assert pcore % 128 == 0
    fg = min(fg, ftot)
    assert ftot % fg == 0 and fg % fc == 0
    ngroups = ftot // fg
    f32 = mybir.dt.float32
    bf16 = mybir.dt.bfloat16
    fp16 = mybir.dt.float16
    i32 = mybir.dt.int32

    nch = C + 1
    ones_col = C

    nc = bacc.Bacc(
        "TRN2", target_bir_lowering=False, debug=False, num_devices=NCORES
    )
    pred_ext = nc.dram_tensor("pred", [C, pcore], f32, kind="ExternalInput")
    lab_ext = nc.dram_tensor("labels", [pcore], i32, kind="ExternalInput")
    iota_ext = nc.dram_tensor("iotarep", [128, K * fc], bf16, kind="ExternalInput")
    out_ext = nc.dram_tensor("out_s", [nch, K], f32, kind="ExternalOutput")

    with tile.TileContext(nc) as tc, ExitStack() as ctx:
        const_pool = ctx.enter_context(tc.tile_pool(name="const", bufs=1))
        slab32_pool = ctx.enter_context(tc.tile_pool(name="slab32", bufs=2))
        slabh_pool = ctx.enter_context(tc.tile_pool(name="slabh", bufs=2))
        lab_pool = ctx.enter_context(tc.tile_pool(name="lab", bufs=2))
        labb_pool = ctx.enter_context(tc.tile_pool(name="labb", bufs=2))
        oh_pool = ctx.enter_context(tc.tile_pool(name="oh", bufs=3))
        psum_pool = ctx.enter_context(tc.tile_pool(name="psum", bufs=1, space="PSUM"))
        out_pool = ctx.enter_context(tc.tile_pool(name="outp", bufs=1))

        iota_t = const_pool.tile([128, K * fc], bf16)
        nc.sync.dma_start(iota_t[:], iota_ext[:])

        psum_t = psum_pool.tile([nch, K], f32)

        nblocks = ftot
        blk = 0
        for g in range(ngroups):
            gpx = 128 * fg
            slab32 = slab32_pool.tile([128, C * fg], f32)
            nc.sync.dma_start(
                slab32[:].rearrange("p (c f) -> p c f", c=C),
                pred_ext[:, g * gpx : (g + 1) * gpx].rearrange(
                    "c (p f) -> p c f", p=128
                ),
            )
            slabh = slabh_pool.tile([128, nch * fg], fp16)
            # scaled fp16 cast on ScalarE: out = Copy(in * 2^14)
            nc.scalar.activation(
                slabh[:, : C * fg],
                slab32[:],
                mybir.ActivationFunctionType.Copy,
                scale=PRED_SCALE,
            )
            nc.vector.memset(slabh[:, ones_col * fg :], 1.0)

            ltile = lab_pool.tile([128, fg], i32)
            nc.sync.dma_start(
                ltile[:],
                lab_ext[g * gpx : (g + 1) * gpx].rearrange("(p f) -> p f", p=128),
            )
            lbt = labb_pool.tile([128, fg], bf16)
            nc.vector.tensor_copy(lbt[:], ltile[:])

            slab_r = slabh[:].rearrange("p (c f) -> p c f", c=nch)
            for ci in range(fg // fc):
                oh = oh_pool.tile([128, K * fc], fp16)
                # j-major: oh[p, j*fc + t] = (labels[p, t] == j+1)
                oh_r = oh[:].rearrange("p (j t) -> p j t", j=K)
                in0 = (
                    lbt[:, ci * fc : (ci + 1) * fc]
                    .unsqueeze(1)
                    .broadcast_to([128, K, fc])
                )
                in1 = iota_t[:].rearrange("p (j t) -> p j t", j=K)
                nc.vector.tensor_tensor(oh_r, in0, in1, mybir.AluOpType.is_equal)
                for t in range(fc):
                    tcol = ci * fc + t
                    nc.tensor.matmul(
                        psum_t[:],
                        slab_r[:, :, tcol],
                        oh[:, t :: fc],
                        start=(blk == 0),
                        stop=(blk == nblocks - 1),
                    )
                    blk += 1

        outt = out_pool.tile([nch, K], f32)
        nc.vector.tensor_copy(outt[:], psum_t[:])
        nc.sync.dma_start(out_ext[:], outt[:])
    nc.compile()
    return nc


def make_iota_np(fc=FC):
    import ml_dtypes

    # j-major repeated iota: value j+1 at [p, j*fc + t]
    v = np.repeat(np.arange(1, K + 1, dtype=np.float32), fc)
    return np.broadcast_to(v, (128, K * fc)).astype(ml_dtypes.bfloat16)


@functools.lru_cache(maxsize=1)
def _get_program():
    return build_nc()


def make_in_maps(pred_flat, labels_flat):
    iota_np = make_iota_np()
    in_maps = []
    for i in range(NCORES):
        sl = slice(i * PCORE, (i + 1) * PCORE)
        in_maps.append(
            {
                "pred": np.ascontiguousarray(pred_flat[:, sl]),
                "labels": np.ascontiguousarray(labels_flat[sl]),
                "iotarep": iota_np,
            }
        )
    return in_maps


def finish_host(parts, num_kernel):
    """parts: list of per-core [9, K] partials. Tiny O(K^2) tail in f64."""
    total = np.sum([p.astype(np.float64) for p in parts], axis=0)
    S = total[:C, :] / PRED_SCALE  # [8, 32]
    N = total[C, :]  # [32]
    A = N * np.sum(S * S, axis=0)  # [32]
    kk = int(num_kernel)
    A = A[:kk]
    pair = A[:, None] + A[None, :]
    Dm = np.maximum(SIGMA_DIS - np.sqrt(pair), 0.0)
    term = np.log(Dm * Dm + 1.0)
    L = float(np.sum(np.triu(term, k=1)))
    L *= (kk - 1) / kk
    return np.float32(L)


_last_results = None


def kernel(pred_similarities, regions_mask, kernel_labels, num_kernel, **kw):
    global _last_results
    from concourse.bass_utils import run_bass_kernel_spmd

    pred_flat = np.asarray(pred_similarities, dtype=np.float32).reshape(C, PTOT)
    labels_flat = np.asarray(kernel_labels, dtype=np.int32).reshape(PTOT)

    nc = _get_program()
    in_maps = make_in_maps(pred_flat, labels_flat)
    res = run_bass_kernel_spmd(nc, in_maps, list(range(NCORES)))
    _last_results = res
    parts = [res.results[i]["out_s"] for i in range(NCORES)]
    return finish_host(parts, num_kernel)


# revision 21
# speedup vs baseline: 2.8778x; 2.6940x over previous
"""Trainium2 Bass kernel for nn_DiscriminationLoss (segment_reduce).

Strategy (8 NeuronCores, pixel-sharded):
  - Each core gets 1/8 of the 4M pixels: pred slice [8, 524288] f32 and
    labels slice [524288] i32.
  - Pixels are tiled [128 partitions x F free]. For each free column t
    (a "block" of 128 pixels), a one-hot matrix oh[p, j] = (labels[p,t]
    == j+1), j in 0..31 is built on DVE (label 0 = background dropped,
    as in the reference). One-hot generation is batched over FC blocks
    per tensor_tensor(is_equal), j-major so all access patterns are
    dense step-1 16-bit (DVE 2x perf mode, ~4.4us per 256-block chunk).
  - pred is scaled by 2^14 and cast to fp16 on ScalarE (the scale rides
    the activation's free affine). fp16 keeps ~2^-11 per-element error;
    the final loss error lands ~1e-5. The host unscales.
  - Per block the PE contracts: psum[9, 32] += st[128, 9]^T @ oh[128, 32]
    (st = 8 scaled-fp16 channels | ones; the ones column yields counts).
    PSUM accumulates over all 4096 blocks per core.
  - GpSimd does nothing: its SBUF port is shared with the DVE and any
    long GpSimd op blocks the DVE one-hot stream (measured 3-4x stalls).
  - Each core emits [9, 32]. Host sums partials over cores (the psum
    step) and evaluates the tiny O(K^2) pairwise tail in f64.
"""

import sys
import functools

sys.path.insert(0, "/opt/trn_rl_repo")

import numpy as np

C = 8
K = 32
NCORES = 8
H = W = 2048
PTOT = H * W
PCORE = PTOT // NCORES  # 524288
SIGMA_DIS = 3.0
PRED_SCALE = float(2.0**14)

FG = 512   # free-dim length per DMA group (128*FG pixels per group)
FC = 256   # free-dim length per one-hot chunk (blocks per tensor_tensor)
QB = 8     # pixel-blocks batched per matmul (block-diagonal trick)
WARM_MMS = 24  # PE warmup matmuls (trip the HAM clock gate to 2.4 GHz)


def build_nc(pcore=PCORE, fg=FG, fc=FC, qb=QB, warm=WARM_MMS):
    import concourse.bacc as bacc
    import concourse.tile as tile
    import concourse.mybir as mybir
    from contextlib import ExitStack

    ftot = pcore // 128
    assert pcore % 128 == 0
    fg = min(fg, ftot)
    assert ftot % fg == 0 and fg % fc == 0
    ngroups = ftot // fg
    f32 = mybir.dt.float32
    bf16 = mybir.dt.bfloat16
    fp16 = mybir.dt.float16
    i32 = mybir.dt.int32

    nch = C + 1
    ones_col = C
    assert fc % qb == 0

    nc = bacc.Bacc(
        "TRN2", target_bir_lowering=False, debug=False, num_devices=NCORES
    )
    pred_ext = nc.dram_tensor("pred", [C, pcore], f32, kind="ExternalInput")
    lab_ext = nc.dram_tensor("labels", [pcore], i32, kind="ExternalInput")
    iota_ext = nc.dram_tensor("iotarep", [128, K * qb], bf16, kind="ExternalInput")
    # rows 0..nch*qb-1: results; row 96: warmup dump (keeps warm MMs live)
    out_ext = nc.dram_tensor("out_s", [128, K * qb], f32, kind="ExternalOutput")

    with tile.TileContext(nc) as tc, ExitStack() as ctx:
        const_pool = ctx.enter_context(tc.tile_pool(name="const", bufs=1))
        slab32_pool = ctx.enter_context(tc.tile_pool(name="slab32", bufs=2))
        slabh_pool = ctx.enter_context(tc.tile_pool(name="slabh", bufs=2))
        lab_pool = ctx.enter_context(tc.tile_pool(name="lab", bufs=2))
        labb_pool = ctx.enter_context(tc.tile_pool(name="labb", bufs=2))
        oh_pool = ctx.enter_context(tc.tile_pool(name="oh", bufs=3))
        psum_pool = ctx.enter_context(tc.tile_pool(name="psum", bufs=1, space="PSUM"))
        out_pool = ctx.enter_context(tc.tile_pool(name="outp", bufs=1))

        iota_t = const_pool.tile([128, K * qb], bf16)
        nc.sync.dma_start(iota_t[:], iota_ext[:])

        psum_full = psum_pool.tile([128, K * qb], f32)
        psum_t = psum_full[: nch * qb, :]

        # PE warmup: ~5us of dense matmuls so the HAM clock gate opens
        # (otherwise every matmul runs at the cold 1.2 GHz rate).
        warm_ps = psum_pool.tile([128, 256], f32)
        if warm:
            for w in range(warm):
                nc.tensor.matmul(
                    warm_ps[:],
                    iota_t[:, :128],
                    iota_t[:, : K * qb],
                    start=(w == 0),
                    stop=(w == warm - 1),
                )

        nblocks = ftot
        blk = 0
        for g in range(ngroups):
            gpx = 128 * fg
            slab32 = slab32_pool.tile([128, C * fg], f32)
            nc.sync.dma_start(
                slab32[:].rearrange("p (c f) -> p c f", c=C),
                pred_ext[:, g * gpx : (g + 1) * gpx].rearrange(
                    "c (p f) -> p c f", p=128
                ),
            )
            # slabh layout: [p, (tg, c, b)], col = tg*(nch*qb) + c*qb + b —
            # each tg-group's stationary [128, nch*qb] is a contiguous slice.
            slabh = slabh_pool.tile([128, nch * fg], fp16)
            slabh_r = slabh[:].rearrange(
                "p (tg c b) -> p tg c b", c=nch, b=qb
            )  # [128, fg/qb, nch, qb]
            slab32_r = slab32[:].rearrange(
                "p (c tg b) -> p tg c b", c=C, b=qb
            )  # in natural (c, t) layout: t = tg*qb + b
            # scaled fp16 cast on ScalarE: out = Copy(in * 2^14)
            nc.scalar.activation(
                slabh_r[:, :, :C, :],
                slab32_r,
                mybir.ActivationFunctionType.Copy,
                scale=PRED_SCALE,
            )
            nc.vector.memset(slabh_r[:, :, ones_col, :], 1.0)

            ltile = lab_pool.tile([128, fg], i32)
            nc.sync.dma_start(
                ltile[:],
                lab_ext[g * gpx : (g + 1) * gpx].rearrange("(p f) -> p f", p=128),
            )
            lbt = labb_pool.tile([128, fg], bf16)
            nc.vector.tensor_copy(lbt[:], ltile[:])

            for ci in range(fg // fc):
                # oh layout: [p, (tg, j, b)] — each tg-group's moving
                # operand [128, K*qb] is a contiguous slice.
                oh = oh_pool.tile([128, K * fc], fp16)
                oh_r = oh[:].rearrange(
                    "p (tg j b) -> p tg j b", j=K, b=qb
                )  # [128, fc/qb, K, qb]
                in0 = (
                    lbt[:, ci * fc : (ci + 1) * fc]
                    .rearrange("p (tg b) -> p tg b", b=qb)
                    .unsqueeze(2)
                    .broadcast_to([128, fc // qb, K, qb])
                )
                in1 = (
                    iota_t[:]
                    .rearrange("p (j b) -> p j b", b=qb)
                    .unsqueeze(1)
                    .broadcast_to([128, fc // qb, K, qb])
                )
                nc.vector.tensor_tensor(oh_r, in0, in1, mybir.AluOpType.is_equal)
                for tg in range(fc // qb):
                    tg_abs = (ci * fc) // qb + tg
                    nc.tensor.matmul(
                        psum_t[:],
                        slabh[:, tg_abs * nch * qb : (tg_abs + 1) * nch * qb],
                        oh[:, tg * K * qb : (tg + 1) * K * qb],
                        start=(blk == 0),
                        stop=(blk == nblocks - qb),
                    )
                    blk += qb

        outt = out_pool.tile([128, K * qb], f32)
        nc.vector.memset(outt[:], 0.0)
        nc.vector.tensor_copy(outt[: nch * qb, :], psum_t[:])
        if warm:
            nc.vector.tensor_copy(outt[96:97, :], warm_ps[96:97, : K * qb])
        nc.sync.dma_start(out_ext[:], outt[:])
    nc.compile()
    return nc


def make_iota_np(qb=QB):
    import ml_dtypes

    # value j+1 at [p, j*qb + b]
    v = np.repeat(np.arange(1, K + 1, dtype=np.float32), qb)
    return np.broadcast_to(v, (128, K * qb)).astype(ml_dtypes.bfloat16)


@functools.lru_cache(maxsize=1)
def _get_program():
    return build_nc()


def make_in_maps(pred_flat, labels_flat):
    iota_np = make_iota_np()
    in_maps = []
    for i in range(NCORES):
        sl = slice(i * PCORE, (i + 1) * PCORE)
        in_maps.append(
            {
                "pred": np.ascontiguousarray(pred_flat[:, sl]),
                "labels": np.ascontiguousarray(labels_flat[sl]),
                "iotarep": iota_np,
            }
        )
    return in_maps


def finish_host(parts, num_kernel, qb=QB):
    """parts: per-core [9*qb+1, K*qb] partials. Tiny O(K^2) tail in f64."""
    nch = C + 1
    total = np.sum([p.astype(np.float64) for p in parts], axis=0)
    r = total[: nch * qb, :].reshape(nch, qb, K, qb)
    total = r[:, np.arange(qb), :, np.arange(qb)].sum(axis=0)  # [nch, K]
    S = total[:C, :] / PRED_SCALE  # [8, 32]
    N = total[C, :]  # [32]
    A = N * np.sum(S * S, axis=0)  # [32]
    kk = int(num_kernel)
    A = A[:kk]
    pair = A[:, None] + A[None, :]
    Dm = np.maximum(SIGMA_DIS - np.sqrt(pair), 0.0)
    term = np.log(Dm * Dm + 1.0)
    L = float(np.sum(np.triu(term, k=1)))
    L *= (kk - 1) / kk
    return np.float32(L)


_last_results = None


def kernel(pred_similarities, regions_mask, kernel_labels, num_kernel, **kw):
    global _last_results
    from concourse.bass_utils import run_bass_kernel_spmd

    pred_flat = np.asarray(pred_similarities, dtype=np.float32).reshape(C, PTOT)
    labels_flat = np.asarray(kernel_labels, dtype=np.int32).reshape(PTOT)

    nc = _get_program()
    in_maps = make_in_maps(pred_flat, labels_flat)
    res = run_bass_kernel_spmd(nc, in_maps, list(range(NCORES)))
    _last_results = res
    parts = [res.results[i]["out_s"] for i in range(NCORES)]
    return finish_host(parts, num_kernel)


# revision 25
# speedup vs baseline: 2.9573x; 1.0276x over previous
"""Trainium2 Bass kernel for nn_DiscriminationLoss (segment_reduce).

Strategy (8 NeuronCores, pixel-sharded):
  - Each core gets 1/8 of the 4M pixels: pred slice [8, 524288] f32 and
    labels slice [524288] i32.
  - Pixels are tiled [128 partitions x F free]. For each free column t
    (a "block" of 128 pixels), a one-hot matrix oh[p, j] = (labels[p,t]
    == j+1), j in 0..31 is built on DVE (label 0 = background dropped,
    as in the reference). One-hot generation is batched over FC blocks
    per tensor_tensor(is_equal), j-major so all access patterns are
    dense step-1 16-bit (DVE 2x perf mode, ~4.4us per 256-block chunk).
  - pred is scaled by 2^14 and cast to fp16 on ScalarE (the scale rides
    the activation's free affine). fp16 keeps ~2^-11 per-element error;
    the final loss error lands ~1e-5. The host unscales.
  - Per block the PE contracts: psum[9, 32] += st[128, 9]^T @ oh[128, 32]
    (st = 8 scaled-fp16 channels | ones; the ones column yields counts).
    PSUM accumulates over all 4096 blocks per core.
  - GpSimd does nothing: its SBUF port is shared with the DVE and any
    long GpSimd op blocks the DVE one-hot stream (measured 3-4x stalls).
  - Each core emits [9, 32]. Host sums partials over cores (the psum
    step) and evaluates the tiny O(K^2) pairwise tail in f64.
"""

import sys
import functools

sys.path.insert(0, "/opt/trn_rl_repo")

import numpy as np

C = 8
K = 32
NCORES = 8
H = W = 2048
PTOT = H * W
PCORE = PTOT // NCORES  # 524288
SIGMA_DIS = 3.0
PRED_SCALE = float(2.0**14)

FG = 512   # free-dim length per DMA group (128*FG pixels per group)
FC = 256   # free-dim length per one-hot chunk (blocks per tensor_tensor)
QB = 8     # pixel-blocks batched per matmul (block-diagonal trick)
WARM_MMS = 40  # PE warmup matmuls (trip the HAM clock gate to 2.4 GHz)


def build_nc(pcore=PCORE, fg=FG, fc=FC, qb=QB, warm=WARM_MMS):
    import concourse.bacc as bacc
    import concourse.tile as tile
    import concourse.mybir as mybir
    from contextlib import ExitStack

    ftot = pcore // 128
    assert pcore % 128 == 0
    fg = min(fg, ftot)
    assert ftot % fg == 0 and fg % fc == 0
    ngroups = ftot // fg
    f32 = mybir.dt.float32
    bf16 = mybir.dt.bfloat16
    fp16 = mybir.dt.float16
    i32 = mybir.dt.int32

    nch = C + 1
    ones_col = C
    assert fc % qb == 0

    nc = bacc.Bacc(
        "TRN2", target_bir_lowering=False, debug=False, num_devices=NCORES
    )
    pred_ext = nc.dram_tensor("pred", [C, pcore], f32, kind="ExternalInput")
    lab_ext = nc.dram_tensor("labels", [pcore], i32, kind="ExternalInput")
    iota_ext = nc.dram_tensor("iotarep", [128, K * qb], bf16, kind="ExternalInput")
    # rows 0..nch*qb-1: results; row 96: warmup dump (keeps warm MMs live)
    out_ext = nc.dram_tensor("out_s", [128, K * qb], f32, kind="ExternalOutput")

    with tile.TileContext(nc) as tc, ExitStack() as ctx:
        const_pool = ctx.enter_context(tc.tile_pool(name="const", bufs=1))
        slab32_pool = ctx.enter_context(tc.tile_pool(name="slab32", bufs=3))
        slabh_pool = ctx.enter_context(tc.tile_pool(name="slabh", bufs=2))
        lab_pool = ctx.enter_context(tc.tile_pool(name="lab", bufs=3))
        labb_pool = ctx.enter_context(tc.tile_pool(name="labb", bufs=2))
        oh_pool = ctx.enter_context(tc.tile_pool(name="oh", bufs=4))
        psum_pool = ctx.enter_context(tc.tile_pool(name="psum", bufs=1, space="PSUM"))
        out_pool = ctx.enter_context(tc.tile_pool(name="outp", bufs=1))

        iota_t = const_pool.tile([128, K * qb], bf16)
        nc.sync.dma_start(iota_t[:], iota_ext[:])

        psum_full = psum_pool.tile([128, K * qb], f32)
        psum_t = psum_full[: nch * qb, :]

        # PE warmup: ~5us of dense matmuls so the HAM clock gate opens
        # (otherwise every matmul runs at the cold 1.2 GHz rate).
        warm_ps = psum_pool.tile([128, 256], f32)
        if warm:
            for w in range(warm):
                nc.tensor.matmul(
                    warm_ps[:],
                    iota_t[:, :128],
                    iota_t[:, : K * qb],
                    start=(w == 0),
                    stop=(w == warm - 1),
                )

        nblocks = ftot
        blk = 0
        for g in range(ngroups):
            gpx = 128 * fg
            slab32 = slab32_pool.tile([128, C * fg], f32)
            nc.sync.dma_start(
                slab32[:].rearrange("p (c f) -> p c f", c=C),
                pred_ext[:, g * gpx : (g + 1) * gpx].rearrange(
                    "c (p f) -> p c f", p=128
                ),
            )
            # slabh layout: [p, (tg, c, b)], col = tg*(nch*qb) + c*qb + b —
            # each tg-group's stationary [128, nch*qb] is a contiguous slice.
            slabh = slabh_pool.tile([128, nch * fg], fp16)
            slabh_r = slabh[:].rearrange(
                "p (tg c b) -> p tg c b", c=nch, b=qb
            )  # [128, fg/qb, nch, qb]
            slab32_r = slab32[:].rearrange(
                "p (c tg b) -> p tg c b", c=C, b=qb
            )  # in natural (c, t) layout: t = tg*qb + b
            # scaled fp16 cast on ScalarE: out = Copy(in * 2^14)
            nc.scalar.activation(
                slabh_r[:, :, :C, :],
                slab32_r,
                mybir.ActivationFunctionType.Copy,
                scale=PRED_SCALE,
            )
            # ones column via ACT: Copy(0*x + 1) = 1.0 (keeps DVE free)
            nc.scalar.activation(
                slabh_r[:, :, ones_col, :],
                slab32[:, : fg],
                mybir.ActivationFunctionType.Copy,
                bias=1.0,
                scale=0.0,
            )

            ltile = lab_pool.tile([128, fg], i32)
            nc.sync.dma_start(
                ltile[:],
                lab_ext[g * gpx : (g + 1) * gpx].rearrange("(p f) -> p f", p=128),
            )
            lbt = labb_pool.tile([128, fg], bf16)
            nc.vector.tensor_copy(lbt[:], ltile[:])

            for ci in range(fg // fc):
                # oh layout: [p, (tg, j, b)] — each tg-group's moving
                # operand [128, K*qb] is a contiguous slice.
                oh = oh_pool.tile([128, K * fc], fp16)
                oh_r = oh[:].rearrange(
                    "p (tg j b) -> p tg j b", j=K, b=qb
                )  # [128, fc/qb, K, qb]
                in0 = (
                    lbt[:, ci * fc : (ci + 1) * fc]
                    .rearrange("p (tg b) -> p tg b", b=qb)
                    .unsqueeze(2)
                    .broadcast_to([128, fc // qb, K, qb])
                )
                in1 = (
                    iota_t[:]
                    .rearrange("p (j b) -> p j b", b=qb)
                    .unsqueeze(1)
                    .broadcast_to([128, fc // qb, K, qb])
                )
                nc.vector.tensor_tensor(oh_r, in0, in1, mybir.AluOpType.is_equal)
                for tg in range(fc // qb):
                    tg_abs = (ci * fc) // qb + tg
                    nc.tensor.matmul(
                        psum_t[:],
                        slabh[:, tg_abs * nch * qb : (tg_abs + 1) * nch * qb],
                        oh[:, tg * K * qb : (tg + 1) * K * qb],
                        start=(blk == 0),
                        stop=(blk == nblocks - qb),
                    )
                    blk += qb

        outt = out_pool.tile([128, K * qb], f32)
        nc.vector.memset(outt[:], 0.0)
        nc.vector.tensor_copy(outt[: nch * qb, :], psum_t[:])
        if warm:
            nc.vector.tensor_copy(outt[96:97, :], warm_ps[96:97, : K * qb])
        nc.sync.dma_start(out_ext[:], outt[:])
    nc.compile()
    return nc


def make_iota_np(qb=QB):
    import ml_dtypes

    # value j+1 at [p, j*qb + b]
    v = np.repeat(np.arange(1, K + 1, dtype=np.float32), qb)
    return np.broadcast_to(v, (128, K * qb)).astype(ml_dtypes.bfloat16)


@functools.lru_cache(maxsize=1)
def _get_program():
    return build_nc()


def make_in_maps(pred_flat, labels_flat):
    iota_np = make_iota_np()
    in_maps = []
    for i in range(NCORES):
        sl = slice(i * PCORE, (i + 1) * PCORE)
        in_maps.append(
            {
                "pred": np.ascontiguousarray(pred_flat[:, sl]),
                "labels": np.ascontiguousarray(labels_flat[sl]),
                "iotarep": iota_np,
            }
        )
    return in_maps


def finish_host(parts, num_kernel, qb=QB):
    """parts: per-core [9*qb+1, K*qb] partials. Tiny O(K^2) tail in f64."""
    nch = C + 1
    total = np.sum([p.astype(np.float64) for p in parts], axis=0)
    r = total[: nch * qb, :].reshape(nch, qb, K, qb)
    total = r[:, np.arange(qb), :, np.arange(qb)].sum(axis=0)  # [nch, K]
    S = total[:C, :] / PRED_SCALE  # [8, 32]
    N = total[C, :]  # [32]
    A = N * np.sum(S * S, axis=0)  # [32]
    kk = int(num_kernel)
    A = A[:kk]
    pair = A[:, None] + A[None, :]
    Dm = np.maximum(SIGMA_DIS - np.sqrt(pair), 0.0)
    term = np.log(Dm * Dm + 1.0)
    L = float(np.sum(np.triu(term, k=1)))
    L *= (kk - 1) / kk
    return np.float32(L)


_last_results = None


def kernel(pred_similarities, regions_mask, kernel_labels, num_kernel, **kw):
    global _last_results
    from concourse.bass_utils import run_bass_kernel_spmd

    pred_flat = np.asarray(pred_similarities, dtype=np.float32).reshape(C, PTOT)
    labels_flat = np.asarray(kernel_labels, dtype=np.int32).reshape(PTOT)

    nc = _get_program()
    in_maps = make_in_maps(pred_flat, labels_flat)
    res = run_bass_kernel_spmd(nc, in_maps, list(range(NCORES)))
    _last_results = res
    parts = [res.results[i]["out_s"] for i in range(NCORES)]
    return finish_host(parts, num_kernel)


# revision 26
# speedup vs baseline: 3.0316x; 1.0251x over previous
"""Trainium2 Bass kernel for nn_DiscriminationLoss (segment_reduce).

Strategy (8 NeuronCores, pixel-sharded):
  - Each core gets 1/8 of the 4M pixels: pred slice [8, 524288] f32 and
    labels slice [524288] i32.
  - Pixels are tiled [128 partitions x F free]. For each free column t
    (a "block" of 128 pixels), a one-hot matrix oh[p, j] = (labels[p,t]
    == j+1), j in 0..31 is built on DVE (label 0 = background dropped,
    as in the reference). One-hot generation is batched over FC blocks
    per tensor_tensor(is_equal), j-major so all access patterns are
    dense step-1 16-bit (DVE 2x perf mode, ~4.4us per 256-block chunk).
  - pred is scaled by 2^14 and cast to fp16 on ScalarE (the scale rides
    the activation's free affine). fp16 keeps ~2^-11 per-element error;
    the final loss error lands ~1e-5. The host unscales.
  - Per block the PE contracts: psum[9, 32] += st[128, 9]^T @ oh[128, 32]
    (st = 8 scaled-fp16 channels | ones; the ones column yields counts).
    PSUM accumulates over all 4096 blocks per core.
  - GpSimd does nothing: its SBUF port is shared with the DVE and any
    long GpSimd op blocks the DVE one-hot stream (measured 3-4x stalls).
  - Each core emits [9, 32]. Host sums partials over cores (the psum
    step) and evaluates the tiny O(K^2) pairwise tail in f64.
"""

import sys
import functools

sys.path.insert(0, "/opt/trn_rl_repo")

import numpy as np

C = 8
K = 32
NCORES = 8
H = W = 2048
PTOT = H * W
PCORE = PTOT // NCORES  # 524288
SIGMA_DIS = 3.0
PRED_SCALE = float(2.0**14)

FG = 256   # free-dim length per DMA group (128*FG pixels per group)
FC = 256   # free-dim length per one-hot chunk (blocks per tensor_tensor)
QB = 8     # pixel-blocks batched per matmul (block-diagonal trick)
WARM_MMS = 40  # PE warmup matmuls (trip the HAM clock gate to 2.4 GHz)


def build_nc(pcore=PCORE, fg=FG, fc=FC, qb=QB, warm=WARM_MMS):
    import concourse.bacc as bacc
    import concourse.tile as tile
    import concourse.mybir as mybir
    from contextlib import ExitStack

    ftot = pcore // 128
    assert pcore % 128 == 0
    fg = min(fg, ftot)
    assert ftot % fg == 0 and fg % fc == 0
    ngroups = ftot // fg
    f32 = mybir.dt.float32
    bf16 = mybir.dt.bfloat16
    fp16 = mybir.dt.float16
    i32 = mybir.dt.int32

    nch = C + 1
    ones_col = C
    assert fc % qb == 0

    nc = bacc.Bacc(
        "TRN2", target_bir_lowering=False, debug=False, num_devices=NCORES
    )
    pred_ext = nc.dram_tensor("pred", [C, pcore], f32, kind="ExternalInput")
    lab_ext = nc.dram_tensor("labels", [pcore], i32, kind="ExternalInput")
    iota_ext = nc.dram_tensor("iotarep", [128, K * qb], bf16, kind="ExternalInput")
    # rows 0..nch*qb-1: results; row 96: warmup dump (keeps warm MMs live)
    out_ext = nc.dram_tensor("out_s", [128, K * qb], f32, kind="ExternalOutput")

    with tile.TileContext(nc) as tc, ExitStack() as ctx:
        const_pool = ctx.enter_context(tc.tile_pool(name="const", bufs=1))
        slab32_pool = ctx.enter_context(tc.tile_pool(name="slab32", bufs=3))
        slabh_pool = ctx.enter_context(tc.tile_pool(name="slabh", bufs=2))
        lab_pool = ctx.enter_context(tc.tile_pool(name="lab", bufs=3))
        labb_pool = ctx.enter_context(tc.tile_pool(name="labb", bufs=2))
        oh_pool = ctx.enter_context(tc.tile_pool(name="oh", bufs=4))
        psum_pool = ctx.enter_context(tc.tile_pool(name="psum", bufs=1, space="PSUM"))
        out_pool = ctx.enter_context(tc.tile_pool(name="outp", bufs=1))

        iota_t = const_pool.tile([128, K * qb], bf16)
        nc.sync.dma_start(iota_t[:], iota_ext[:])

        psum_full = psum_pool.tile([128, K * qb], f32)
        psum_t = psum_full[: nch * qb, :]

        # PE warmup: ~5us of dense matmuls so the HAM clock gate opens
        # (otherwise every matmul runs at the cold 1.2 GHz rate).
        warm_ps = psum_pool.tile([128, 256], f32)
        if warm:
            for w in range(warm):
                nc.tensor.matmul(
                    warm_ps[:],
                    iota_t[:, :128],
                    iota_t[:, : K * qb],
                    start=(w == 0),
                    stop=(w == warm - 1),
                )

        nblocks = ftot
        blk = 0
        for g in range(ngroups):
            gpx = 128 * fg
            slab32 = slab32_pool.tile([128, C * fg], f32)
            nc.sync.dma_start(
                slab32[:].rearrange("p (c f) -> p c f", c=C),
                pred_ext[:, g * gpx : (g + 1) * gpx].rearrange(
                    "c (p f) -> p c f", p=128
                ),
            )
            # slabh layout: [p, (tg, c, b)], col = tg*(nch*qb) + c*qb + b —
            # each tg-group's stationary [128, nch*qb] is a contiguous slice.
            slabh = slabh_pool.tile([128, nch * fg], fp16)
            slabh_r = slabh[:].rearrange(
                "p (tg c b) -> p tg c b", c=nch, b=qb
            )  # [128, fg/qb, nch, qb]
            slab32_r = slab32[:].rearrange(
                "p (c tg b) -> p tg c b", c=C, b=qb
            )  # in natural (c, t) layout: t = tg*qb + b
            # scaled fp16 cast on ScalarE: out = Copy(in * 2^14)
            nc.scalar.activation(
                slabh_r[:, :, :C, :],
                slab32_r,
                mybir.ActivationFunctionType.Copy,
                scale=PRED_SCALE,
            )
            # ones column via ACT: Copy(0*x + 1) = 1.0 (keeps DVE free)
            nc.scalar.activation(
                slabh_r[:, :, ones_col, :],
                slab32[:, : fg],
                mybir.ActivationFunctionType.Copy,
                bias=1.0,
                scale=0.0,
            )

            ltile = lab_pool.tile([128, fg], i32)
            nc.sync.dma_start(
                ltile[:],
                lab_ext[g * gpx : (g + 1) * gpx].rearrange("(p f) -> p f", p=128),
            )
            lbt = labb_pool.tile([128, fg], bf16)
            nc.vector.tensor_copy(lbt[:], ltile[:])

            for ci in range(fg // fc):
                # oh layout: [p, (tg, j, b)] — each tg-group's moving
                # operand [128, K*qb] is a contiguous slice.
                oh = oh_pool.tile([128, K * fc], fp16)
                oh_r = oh[:].rearrange(
                    "p (tg j b) -> p tg j b", j=K, b=qb
                )  # [128, fc/qb, K, qb]
                in0 = (
                    lbt[:, ci * fc : (ci + 1) * fc]
                    .rearrange("p (tg b) -> p tg b", b=qb)
                    .unsqueeze(2)
                    .broadcast_to([128, fc // qb, K, qb])
                )
                in1 = (
                    iota_t[:]
                    .rearrange("p (j b) -> p j b", b=qb)
                    .unsqueeze(1)
                    .broadcast_to([128, fc // qb, K, qb])
                )
                nc.vector.tensor_tensor(oh_r, in0, in1, mybir.AluOpType.is_equal)
                for tg in range(fc // qb):
                    tg_abs = (ci * fc) // qb + tg
                    nc.tensor.matmul(
                        psum_t[:],
                        slabh[:, tg_abs * nch * qb : (tg_abs + 1) * nch * qb],
                        oh[:, tg * K * qb : (tg + 1) * K * qb],
                        start=(blk == 0),
                        stop=(blk == nblocks - qb),
                    )
                    blk += qb

        outt = out_pool.tile([128, K * qb], f32)
        nc.vector.memset(outt[:], 0.0)
        nc.vector.tensor_copy(outt[: nch * qb, :], psum_t[:])
        if warm:
            nc.vector.tensor_copy(outt[96:97, :], warm_ps[96:97, : K * qb])
        nc.sync.dma_start(out_ext[:], outt[:])
    nc.compile()
    return nc


def make_iota_np(qb=QB):
    import ml_dtypes

    # value j+1 at [p, j*qb + b]
    v = np.repeat(np.arange(1, K + 1, dtype=np.float32), qb)
    return np.broadcast_to(v, (128, K * qb)).astype(ml_dtypes.bfloat16)


@functools.lru_cache(maxsize=1)
def _get_program():
    return build_nc()


def make_in_maps(pred_flat, labels_flat):
    iota_np = make_iota_np()
    in_maps = []
    for i in range(NCORES):
        sl = slice(i * PCORE, (i + 1) * PCORE)
        in_maps.append(
            {
                "pred": np.ascontiguousarray(pred_flat[:, sl]),
                "labels": np.ascontiguousarray(labels_flat[sl]),
                "iotarep": iota_np,
            }
        )
    return in_maps


def finish_host(parts, num_kernel, qb=QB):
    """parts: per-core [9*qb+1, K*qb] partials. Tiny O(K^2) tail in f64."""
    nch = C + 1
    total = np.sum([p.astype(np.float64) for p in parts], axis=0)
    r = total[: nch * qb, :].reshape(nch, qb, K, qb)
    total = r[:, np.arange(qb), :, np.arange(qb)].sum(axis=0)  # [nch, K]
    S = total[:C, :] / PRED_SCALE  # [8, 32]
    N = total[C, :]  # [32]
    A = N * np.sum(S * S, axis=0)  # [32]
    kk = int(num_kernel)
    A = A[:kk]
    pair = A[:, None] + A[None, :]
    Dm = np.maximum(SIGMA_DIS - np.sqrt(pair), 0.0)
    term = np.log(Dm * Dm + 1.0)
    L = float(np.sum(np.triu(term, k=1)))
    L *= (kk - 1) / kk
    return np.float32(L)


_last_results = None


def kernel(pred_similarities, regions_mask, kernel_labels, num_kernel, **kw):
    global _last_results
    from concourse.bass_utils import run_bass_kernel_spmd

    pred_flat = np.asarray(pred_similarities, dtype=np.float32).reshape(C, PTOT)
    labels_flat = np.asarray(kernel_labels, dtype=np.int32).reshape(PTOT)

    nc = _get_program()
    in_maps = make_in_maps(pred_flat, labels_flat)
    res = run_bass_kernel_spmd(nc, in_maps, list(range(NCORES)))
    _last_results = res
    parts = [res.results[i]["out_s"] for i in range(NCORES)]
    return finish_host(parts, num_kernel)


# revision 29
# speedup vs baseline: 3.2102x; 1.0589x over previous
"""Trainium2 Bass kernel for nn_DiscriminationLoss (segment_reduce).

Strategy (8 NeuronCores, pixel-sharded):
  - Each core gets 1/8 of the 4M pixels: pred slice [8, 524288] f32 and
    labels slice [524288] i32.
  - Pixels are tiled [128 partitions x F free]. For each free column t
    (a "block" of 128 pixels), a one-hot matrix oh[p, j] = (labels[p,t]
    == j+1), j in 0..31 is built on DVE (label 0 = background dropped,
    as in the reference). One-hot generation is batched over FC blocks
    per tensor_tensor(is_equal), j-major so all access patterns are
    dense step-1 16-bit (DVE 2x perf mode, ~4.4us per 256-block chunk).
  - pred is scaled by 2^14 and cast to fp16 on ScalarE (the scale rides
    the activation's free affine). fp16 keeps ~2^-11 per-element error;
    the final loss error lands ~1e-5. The host unscales.
  - Per block the PE contracts: psum[9, 32] += st[128, 9]^T @ oh[128, 32]
    (st = 8 scaled-fp16 channels | ones; the ones column yields counts).
    PSUM accumulates over all 4096 blocks per core.
  - GpSimd does nothing: its SBUF port is shared with the DVE and any
    long GpSimd op blocks the DVE one-hot stream (measured 3-4x stalls).
  - Each core emits [9, 32]. Host sums partials over cores (the psum
    step) and evaluates the tiny O(K^2) pairwise tail in f64.
"""

import sys
import functools

sys.path.insert(0, "/opt/trn_rl_repo")

import numpy as np

C = 8
K = 32
NCORES = 8
H = W = 2048
PTOT = H * W
PCORE = PTOT // NCORES  # 524288
SIGMA_DIS = 3.0
PRED_SCALE = float(2.0**14)

FG = 256   # free-dim length per DMA group (128*FG pixels per group)
FC = 256   # free-dim length per one-hot chunk (blocks per tensor_tensor)
QB = 8     # pixel-blocks batched per matmul (block-diagonal trick)
WARM_MMS = 40  # PE warmup matmuls (trip the HAM clock gate to 2.4 GHz)


def build_nc(pcore=PCORE, fg=FG, fc=FC, qb=QB, warm=WARM_MMS):
    import concourse.bacc as bacc
    import concourse.tile as tile
    import concourse.mybir as mybir
    from contextlib import ExitStack

    ftot = pcore // 128
    assert pcore % 128 == 0
    fg = min(fg, ftot)
    assert ftot % fg == 0 and fg % fc == 0
    # small leading groups prime the pipeline faster
    if ftot >= 8 * fg and fg >= 2 * qb and fg % 2 == 0:
        group_sizes = [fg // 2, fg // 2] + [fg] * (ftot // fg - 1)
    else:
        group_sizes = [fg] * (ftot // fg)
    assert sum(group_sizes) == ftot
    f32 = mybir.dt.float32
    bf16 = mybir.dt.bfloat16
    fp16 = mybir.dt.float16
    i32 = mybir.dt.int32

    nch = C + 1
    ones_col = C
    assert fc % qb == 0

    nc = bacc.Bacc(
        "TRN2", target_bir_lowering=False, debug=False, num_devices=NCORES
    )
    pred_ext = nc.dram_tensor("pred", [C, pcore], f32, kind="ExternalInput")
    lab_ext = nc.dram_tensor("labels", [pcore], i32, kind="ExternalInput")
    iota_ext = nc.dram_tensor("iotarep", [128, K * qb], bf16, kind="ExternalInput")
    # rows 0..nch*qb-1: results; row 96: warmup dump (keeps warm MMs live)
    out_ext = nc.dram_tensor("out_s", [128, K * qb], f32, kind="ExternalOutput")

    with tile.TileContext(nc) as tc, ExitStack() as ctx:
        const_pool = ctx.enter_context(tc.tile_pool(name="const", bufs=1))
        slab32_pool = ctx.enter_context(tc.tile_pool(name="slab32", bufs=3))
        slabh_pool = ctx.enter_context(tc.tile_pool(name="slabh", bufs=2))
        labb_pool = ctx.enter_context(tc.tile_pool(name="labb", bufs=3))
        oh_pool = ctx.enter_context(tc.tile_pool(name="oh", bufs=6))
        psum_pool = ctx.enter_context(tc.tile_pool(name="psum", bufs=1, space="PSUM"))
        out_pool = ctx.enter_context(tc.tile_pool(name="outp", bufs=1))

        iota_t = const_pool.tile([128, K * qb], bf16)
        nc.sync.dma_start(iota_t[:], iota_ext[:])

        psum_full = psum_pool.tile([128, K * qb], f32)
        psum_t = psum_full[: nch * qb, :]

        # PE warmup: ~5us of dense matmuls so the HAM clock gate opens
        # (otherwise every matmul runs at the cold 1.2 GHz rate).
        warm_ps = psum_pool.tile([128, 256], f32)
        if warm:
            for w in range(warm):
                nc.tensor.matmul(
                    warm_ps[:],
                    iota_t[:, :128],
                    iota_t[:, : K * qb],
                    start=(w == 0),
                    stop=(w == warm - 1),
                )

        nblocks = ftot
        blk = 0
        goff = 0
        for fgg in group_sizes:
            gpx = 128 * fgg
            poff = 128 * goff
            slab32 = slab32_pool.tile([128, C * fg], f32, tag="slab32")
            s32 = slab32[:, : C * fgg]
            nc.sync.dma_start(
                s32.rearrange("p (c f) -> p c f", c=C),
                pred_ext[:, poff : poff + gpx].rearrange("c (p f) -> p c f", p=128),
            )
            # slabh layout: [p, (tg, c, b)], col = tg*(nch*qb) + c*qb + b —
            # each tg-group's stationary [128, nch*qb] is a contiguous slice.
            slabh = slabh_pool.tile([128, nch * fg], fp16, tag="slabh")
            slabh_r = slabh[:, : nch * fgg].rearrange(
                "p (tg c b) -> p tg c b", c=nch, b=qb
            )  # [128, fgg/qb, nch, qb]
            slab32_r = s32.rearrange(
                "p (c tg b) -> p tg c b", c=C, b=qb
            )  # in natural (c, t) layout: t = tg*qb + b
            # scaled fp16 cast on ScalarE: out = Copy(in * 2^14)
            nc.scalar.activation(
                slabh_r[:, :, :C, :],
                slab32_r,
                mybir.ActivationFunctionType.Copy,
                scale=PRED_SCALE,
            )
            # ones column via ACT: Copy(0*x + 1) = 1.0 (keeps DVE free)
            nc.scalar.activation(
                slabh_r[:, :, ones_col, :],
                s32[:, :fgg],
                mybir.ActivationFunctionType.Copy,
                bias=1.0,
                scale=0.0,
            )

            # labels arrive as bf16 via SWDGE cast-DMA (no DVE copy needed)
            lbt = labb_pool.tile([128, fg], bf16, tag="labb")
            nc.gpsimd.dma_start(
                lbt[:, :fgg],
                lab_ext[poff : poff + gpx].rearrange("(p f) -> p f", p=128),
            )

            fcg = min(fc, fgg)
            for ci in range(fgg // fcg):
                # oh layout: [p, (tg, j, b)] — each tg-group's moving
                # operand [128, K*qb] is a contiguous slice.
                oh = oh_pool.tile([128, K * fc], fp16, tag="oh")
                oh_r = oh[:, : K * fcg].rearrange(
                    "p (tg j b) -> p tg j b", j=K, b=qb
                )  # [128, fcg/qb, K, qb]
                in0 = (
                    lbt[:, ci * fcg : (ci + 1) * fcg]
                    .rearrange("p (tg b) -> p tg b", b=qb)
                    .unsqueeze(2)
                    .broadcast_to([128, fcg // qb, K, qb])
                )
                in1 = (
                    iota_t[:]
                    .rearrange("p (j b) -> p j b", b=qb)
                    .unsqueeze(1)
                    .broadcast_to([128, fcg // qb, K, qb])
                )
                nc.vector.tensor_tensor(oh_r, in0, in1, mybir.AluOpType.is_equal)
                for tg in range(fcg // qb):
                    tg_abs = (ci * fcg) // qb + tg
                    nc.tensor.matmul(
                        psum_t[:],
                        slabh[:, tg_abs * nch * qb : (tg_abs + 1) * nch * qb],
                        oh[:, tg * K * qb : (tg + 1) * K * qb],
                        start=(blk == 0),
                        stop=(blk == nblocks - qb),
                    )
                    blk += qb
            goff += fgg

        outt = out_pool.tile([128, K * qb], f32)
        nc.vector.memset(outt[:], 0.0)
        nc.vector.tensor_copy(outt[: nch * qb, :], psum_t[:])
        if warm:
            nc.vector.tensor_copy(outt[96:97, :], warm_ps[96:97, : K * qb])
        nc.sync.dma_start(out_ext[:], outt[:])
    nc.compile()
    return nc


def make_iota_np(qb=QB):
    import ml_dtypes

    # value j+1 at [p, j*qb + b]
    v = np.repeat(np.arange(1, K + 1, dtype=np.float32), qb)
    return np.broadcast_to(v, (128, K * qb)).astype(ml_dtypes.bfloat16)


@functools.lru_cache(maxsize=1)
def _get_program():
    return build_nc()


def make_in_maps(pred_flat, labels_flat):
    iota_np = make_iota_np()
    in_maps = []
    for i in range(NCORES):
        sl = slice(i * PCORE, (i + 1) * PCORE)
        in_maps.append(
            {
                "pred": np.ascontiguousarray(pred_flat[:, sl]),
                "labels": np.ascontiguousarray(labels_flat[sl]),
                "iotarep": iota_np,
            }
        )
    return in_maps


def finish_host(parts, num_kernel, qb=QB):
    """parts: per-core [9*qb+1, K*qb] partials. Tiny O(K^2) tail in f64."""
    nch = C + 1
    total = np.sum([p.astype(np.float64) for p in parts], axis=0)
    r = total[: nch * qb, :].reshape(nch, qb, K, qb)
    total = r[:, np.arange(qb), :, np.arange(qb)].sum(axis=0)  # [nch, K]
    S = total[:C, :] / PRED_SCALE  # [8, 32]
    N = total[C, :]  # [32]
    A = N * np.sum(S * S, axis=0)  # [32]
    kk = int(num_kernel)
    A = A[:kk]
    pair = A[:, None] + A[None, :]
    Dm = np.maximum(SIGMA_DIS - np.sqrt(pair), 0.0)
    term = np.log(Dm * Dm + 1.0)
    L = float(np.sum(np.triu(term, k=1)))
    L *= (kk - 1) / kk
    return np.float32(L)


_last_results = None


def kernel(pred_similarities, regions_mask, kernel_labels, num_kernel, **kw):
    global _last_results
    from concourse.bass_utils import run_bass_kernel_spmd

    pred_flat = np.asarray(pred_similarities, dtype=np.float32).reshape(C, PTOT)
    labels_flat = np.asarray(kernel_labels, dtype=np.int32).reshape(PTOT)

    nc = _get_program()
    in_maps = make_in_maps(pred_flat, labels_flat)
    res = run_bass_kernel_spmd(nc, in_maps, list(range(NCORES)))
    _last_results = res
    parts = [res.results[i]["out_s"] for i in range(NCORES)]
    return finish_host(parts, num_kernel)
